# revision 13
# baseline (speedup 1.0000x reference)
"""Trainium2 Bass kernel for nn_LiquidLoRALayer.

Computation (forward only; see problem reference):
    hidden <- 3 liquid-dynamics steps on [O, r] state (target = lora_B)
    B_eff   = hidden (the straight-through trick is a numeric no-op)
    out     = (x @ (2*lora_A)^T) @ B_eff^T          # SCALING=2 folded into A

Sharding: data-parallel over the B*S=16384 rows across 8 cores (2048 rows
per core); all small parameters replicated.

The kernel is HBM-bandwidth bound (per-core cap ~360 GB/s with all 8
cores pulling), so both the x shard and the output travel as fp16
(16.8 MB each per core); measured end-to-end rel err of the fp16 scheme
is ~2.4e-3 against the f32 reference.

PE usage: both big matmul stages run as concurrent PE tile pairs
(tile_position derived from operand/output base partitions):
  stage 1  tt = (2A) @ x^T    two (128x64) column tiles, one per half of
                              each 512-row x block
  stage 2  out = tt^T @ Beff  two (64x128) row tiles, lo/hi halves of
                              tt2/beff stacked on partitions 0-63/64-127
The liquid chain runs in fp16 on a packed [128, O/2] layout with work
split across ACT (sigmoid/exp), DVE (reciprocals/adds) and GpSimd
(muls/sub); stage-2 PSUM drains rotate across all three engines.
"""

import numpy as np
from contextlib import ExitStack

# Problem shapes (hardcoded per spec).
B_, S_, D_, O_, R_ = 4, 4096, 4096, 4096, 64
N_CORES = 8
M_TOTAL = B_ * S_
M_CORE = M_TOTAL // N_CORES

SCALING = 128.0 / 64.0
DT_STEP = 0.1
TAU_MIN = 0.1
TAU_MAX = 10.0
ADAPT_STEPS = 3

LAST_RESULTS = None  # stashed BassKernelResults from the most recent run


def build_nc(D, O, M, R=64, M_BLK=512):
    """Build the per-core Bass program. All 8 cores run this same program
    on different `xt` shards."""
    import concourse.bacc as bacc
    import concourse.tile as tile
    import concourse.mybir as mybir

    f32 = mybir.dt.float32
    f16 = mybir.dt.float16
    AF = mybir.ActivationFunctionType
    ALU = mybir.AluOpType

    DC = D // 128        # contraction chunks
    OH = O // 2          # packed-half width
    NB = M // M_BLK      # row blocks per core
    HB = M_BLK // 2      # half-block rows (one PE column tile each)
    OC = O // 512        # output column chunks
    BW = DC * M_BLK      # columns per x block in the packed layout

    LW = 4 * R

    nc = bacc.Bacc()
    xt = nc.dram_tensor("xt", [128, NB * BW], f16, kind="ExternalInput")
    at2p = nc.dram_tensor("at2p", [128, DC * R], f16, kind="ExternalInput")
    lparams = nc.dram_tensor("lparams", [128, LW], f16, kind="ExternalInput")
    ldata = nc.dram_tensor("ldata", [128, 2 * OH], f16, kind="ExternalInput")
    sparams = nc.dram_tensor("sparams", [128, 2], f32, kind="ExternalInput")
    out = nc.dram_tensor("out", [M, O], f16, kind="ExternalOutput")

    with tile.TileContext(nc) as tc, ExitStack() as ctx:
        const = ctx.enter_context(tc.tile_pool(name="const", bufs=1))
        lqf = ctx.enter_context(tc.tile_pool(name="lqf", bufs=5))
        lqh = ctx.enter_context(tc.tile_pool(name="lqh", bufs=8))
        hpool = ctx.enter_context(tc.tile_pool(name="hbuf", bufs=2))
        xtp = ctx.enter_context(tc.tile_pool(name="xtp", bufs=3))
        outp = ctx.enter_context(tc.tile_pool(name="outp", bufs=4))
        ps_tt = ctx.enter_context(tc.tile_pool(name="ps_tt", bufs=1, space="PSUM"))
        ps_out = ctx.enter_context(tc.tile_pool(name="ps_out", bufs=5, space="PSUM"))
        ps_pre = ctx.enter_context(tc.tile_pool(name="ps_pre", bufs=2, space="PSUM"))

        # ---- params ---------------------------------------------------------
        # everything the liquid needs rides the sync queue AHEAD of the x
        # blocks so the liquid chain starts within a few us.
        lpa = const.tile([128, LW], f16)
        nc.sync.dma_start(out=lpa, in_=lparams[:, :])
        lw_gt = lpa[:, 0:R]
        lw_gh = lpa[:, R:2 * R]
        lw_tt = lpa[:, 2 * R:3 * R]
        lw_th = lpa[:, 3 * R:4 * R]
        spa = const.tile([128, 2], f32)
        nc.sync.dma_start(out=spa, in_=sparams[:, :])
        bgd_ap = spa[:, 0:1]
        btd_ap = spa[:, 1:2]
        lda = const.tile([128, 2 * OH], f16)
        nc.sync.dma_start(out=lda, in_=ldata[:, :])
        btp = lda[:, 0:OH]
        h0 = lda[:, OH:2 * OH]
        pa2 = const.tile([128, DC * R], f16)
        nc.sync.dma_start(out=pa2, in_=at2p[:, :])

        def at2_ap(c):
            return pa2[:, c * R:(c + 1) * R]

        tt2 = const.tile([128, NB * HB], f16)   # lo/hi tt halves stacked
        beff = const.tile([128, O], f16)        # B_eff^T duplicated lo/hi

        hst = {"h": h0}

        # ---- liquid dynamics (replicated on every core) ---------------------
        # State packed [128, OH]: partition p<64 -> (r=p, o<OH), p>=64 ->
        # (r=p-64, o>=OH). One fp16 state tile per step serves both the gate
        # matmuls and the elementwise chain. Work split: ACT sigmoid/exp,
        # DVE tensor_scalar/reciprocal/add, GpSimd muls+sub.
        CH = 1024
        NCH = OH // CH

        def liquid_step(step):
            h_cur = hst["h"]
            h_new = hpool.tile([128, OH], f16, tag="h", name=f"h{step}")
            # phase 1: all sigmoids for both chunks back-to-back (one
            # SIGMOID table load per step; EXPs later share one EXP load)
            sfs, sts = [], []
            for ch in range(NCH):
                s_f = lqh.tile([128, CH], f16, tag="lqh", name=f"sf{step}_{ch}")
                s_t = lqh.tile([128, CH], f16, tag="lqh", name=f"st{step}_{ch}")
                sfs.append(s_f)
                sts.append(s_t)
                for w_t, w_h, bias_ap, s_out in (
                    (lw_gt, lw_gh, bgd_ap, s_f),
                    (lw_tt, lw_th, btd_ap, s_t),
                ):
                    for j in range(CH // 512):
                        pre = ps_pre.tile([128, 512], f32, tag="pre",
                                          name=f"pre{step}_{ch}_{j}")
                        jsl = slice(ch * CH + j * 512, ch * CH + (j + 1) * 512)
                        for hb in (0, 1):
                            sl = slice(64 * hb, 64 * hb + 64)
                            # contraction split: target rows then h rows
                            nc.tensor.matmul(
                                pre[sl, :], lhsT=w_t[sl, :],
                                rhs=btp[sl, jsl], start=True, stop=False)
                            nc.tensor.matmul(
                                pre[sl, :], lhsT=w_h[sl, :],
                                rhs=h_cur[sl, jsl], start=False, stop=True)
                        nc.scalar.activation(
                            out=s_out[:, j * 512:(j + 1) * 512], in_=pre[:, :],
                            func=AF.Sigmoid, bias=bias_ap, scale=1.0)
            # phase 2: elementwise chain per chunk.
            # DVE: affine/recip/muls; GpSimd: g and d; ACT: exp.
            for ch in range(NCH):
                csl = slice(ch * CH, (ch + 1) * CH)
                s_f, s_t = sfs[ch], sts[ch]
                tau = lqf.tile([128, CH], f32, tag="lqf", name=f"tau{step}_{ch}")
                nc.vector.tensor_scalar(tau, s_t, TAU_MAX - TAU_MIN, TAU_MIN,
                                        ALU.mult, ALU.add)
                rt = lqf.tile([128, CH], f32, tag="lqf", name=f"rt{step}_{ch}")
                nc.vector.reciprocal_approx_fast(out=rt, in_=tau)
                g = lqh.tile([128, CH], f16, tag="lqh", name=f"g{step}_{ch}")
                nc.gpsimd.tensor_mul(g, s_f, btp[:, csl])
                a = lqf.tile([128, CH], f32, tag="lqf", name=f"a{step}_{ch}")
                nc.vector.tensor_add(a, rt, s_f)
                ra = lqf.tile([128, CH], f32, tag="lqf", name=f"ra{step}_{ch}")
                nc.vector.reciprocal_approx_fast(out=ra, in_=a)
                e = lqh.tile([128, CH], f16, tag="lqh", name=f"e{step}_{ch}")
                nc.scalar.activation(out=e, in_=a, func=AF.Exp, scale=-DT_STEP)
                p_ = lqh.tile([128, CH], f16, tag="lqh", name=f"p{step}_{ch}")
                nc.gpsimd.tensor_mul(p_, ra, g)   # (f/a) * target
                d_ = lqh.tile([128, CH], f16, tag="lqh", name=f"d{step}_{ch}")
                nc.gpsimd.tensor_sub(d_, h_cur[:, csl], p_)
                de = lqh.tile([128, CH], f16, tag="lqh", name=f"de{step}_{ch}")
                nc.vector.tensor_mul(de, d_, e)
                nc.vector.tensor_add(h_new[:, csl], de, p_)
            hst["h"] = h_new

        # ---- main pipeline stage 1: tt = (2A) @ x^T -------------------------
        # Each 512-row block is split into two 256-row halves that run on the
        # two 128x64 PE column tiles concurrently (out partition base 0/64).
        def in_chain(b):
            xt_sb = xtp.tile([128, DC, M_BLK], f16, tag="xt", name=f"xt_sb{b}")
            nc.sync.dma_start(
                out=xt_sb,
                in_=xt[:, b * BW:(b + 1) * BW].rearrange(
                    "p (c m) -> p c m", c=DC))
            tt_ps = ps_tt.tile([128, HB], f32, tag="tt_ps", name=f"tt_ps{b}")
            for c in range(DC):
                nc.tensor.matmul(
                    tt_ps[0:64, :], lhsT=at2_ap(c), rhs=xt_sb[:, c, 0:HB],
                    start=(c == 0), stop=(c == DC - 1))
                nc.tensor.matmul(
                    tt_ps[64:128, :], lhsT=at2_ap(c), rhs=xt_sb[:, c, HB:M_BLK],
                    start=(c == 0), stop=(c == DC - 1))
            nc.scalar.copy(out=tt2[:, b * HB:(b + 1) * HB], in_=tt_ps)

        # ---- main pipeline stage 2: out = tt @ B_eff^T ----------------------
        # lo/hi subtiles run on the two 64x128 PE row tiles concurrently
        # (operand partition base 0/64). Each PSUM tile takes two adjacent
        # 512-col matmuls and drains with one [128,1024] copy, alternating
        # DVE/ACT (GpSimd cannot read PSUM).
        def out_chain(b):
            for ms in range(HB // 128):
                o_lo = outp.tile([128, O], f16, tag="osb", name=f"olo{b}_{ms}")
                o_hi = outp.tile([128, O], f16, tag="osb", name=f"ohi{b}_{ms}")
                msl = slice(b * HB + ms * 128, b * HB + (ms + 1) * 128)
                for oc in range(OC):
                    osl = slice(oc * 512, (oc + 1) * 512)
                    op_l = ps_out.tile([128, 512], f32, tag="op",
                                       name=f"opl{b}_{ms}_{oc}")
                    op_h = ps_out.tile([128, 512], f32, tag="op",
                                       name=f"oph{b}_{ms}_{oc}")
                    nc.tensor.matmul(op_l, lhsT=tt2[0:64, msl],
                                     rhs=beff[0:64, osl],
                                     start=True, stop=True)
                    nc.tensor.matmul(op_h, lhsT=tt2[64:128, msl],
                                     rhs=beff[64:128, osl],
                                     start=True, stop=True)
                    if oc % 2 == 0:
                        nc.vector.tensor_copy(out=o_lo[:, osl], in_=op_l)
                        nc.scalar.copy(out=o_hi[:, osl], in_=op_h)
                    else:
                        nc.scalar.copy(out=o_lo[:, osl], in_=op_l)
                        nc.vector.tensor_copy(out=o_hi[:, osl], in_=op_h)
                r_lo = b * M_BLK + ms * 128
                r_hi = b * M_BLK + HB + ms * 128
                # output rides the sync HWDGE queue: the sync engine is idle
                # once the input blocks (all issued earlier) are in flight
                nc.sync.dma_start(out=out[r_lo:r_lo + 128, :], in_=o_lo)
                nc.sync.dma_start(out=out[r_hi:r_hi + 128, :], in_=o_hi)

        # ---- driver ---------------------------------------------------------
        for step in range(ADAPT_STEPS):
            liquid_step(step)
            in_chain(step)

        # unpack B_eff^T to [128, O] (fp16), duplicated on partition halves:
        # aligned halves via DVE/ACT copies, crossed halves via SBUF DMAs
        h3 = hst["h"]
        nc.vector.tensor_copy(out=beff[0:64, 0:OH], in_=h3[0:64, :])
        nc.scalar.copy(out=beff[64:128, OH:O], in_=h3[64:128, :])
        nc.gpsimd.dma_start(out=beff[0:64, OH:O], in_=h3[64:128, :])
        nc.gpsimd.dma_start(out=beff[64:128, 0:OH], in_=h3[0:64, :])

        in_chain(3)
        out_chain(0)
        out_chain(1)
        out_chain(2)
        out_chain(3)
    nc.finalize()
    return nc


def make_host_inputs(x, lora_A, lora_B, hidden_B, W_gate, b_gate, W_tau, b_tau,
                     n_cores=N_CORES, M_BLK=512):
    """Host-side sharding / layout prep. Returns the per-core in_maps."""
    f16 = np.float16

    x = np.asarray(x, dtype=np.float32)
    M = x.shape[0] * x.shape[1] if x.ndim == 3 else x.shape[0]
    D = x.shape[-1]
    O = lora_B.shape[0]
    R = lora_B.shape[1]
    OH = O // 2
    DC = D // 128
    Mc = M // n_cores
    NB = Mc // M_BLK
    x2 = x.reshape(M, D)

    BT = np.asarray(lora_B, np.float32).T                    # [r, O]
    btp_np = np.concatenate([BT[:, :OH], BT[:, OH:]], axis=0)  # [128, OH]
    hT = np.asarray(hidden_B, np.float32).T
    h0p_np = np.concatenate([hT[:, :OH], hT[:, OH:]], axis=0)
    WgT = np.asarray(W_gate, np.float32).T                   # [2r, r]
    WtT = np.asarray(W_tau, np.float32).T
    wgt_np = np.concatenate([WgT[:R], WgT[:R]], axis=0)      # [128, r]
    wgh_np = np.concatenate([WgT[R:], WgT[R:]], axis=0)
    wtt_np = np.concatenate([WtT[:R], WtT[:R]], axis=0)
    wth_np = np.concatenate([WtT[R:], WtT[R:]], axis=0)
    bg = np.asarray(b_gate, np.float32)
    bt = np.asarray(b_tau, np.float32)
    bgd_np = np.concatenate([bg, bg]).reshape(128, 1)
    btd_np = np.concatenate([bt, bt]).reshape(128, 1)
    at2 = (2.0 * np.asarray(lora_A, np.float32)).T           # [D, r]
    # at2 packed as [128, DC*r]: column block c = rows c*128..c*128+128
    at2_pk = at2.reshape(DC, 128, R).transpose(1, 0, 2).reshape(128, DC * R)

    lparams_np = np.ascontiguousarray(np.concatenate(
        [wgt_np, wgh_np, wtt_np, wth_np], axis=1).astype(f16))
    ldata_np = np.ascontiguousarray(np.concatenate(
        [btp_np, h0p_np], axis=1).astype(f16))
    sparams_np = np.ascontiguousarray(
        np.concatenate([bgd_np, btd_np], axis=1))
    at2p_np = np.ascontiguousarray(at2_pk.astype(f16))

    shared = dict(at2p=at2p_np, lparams=lparams_np,
                  ldata=ldata_np, sparams=sparams_np)
    in_maps = []
    for c in range(n_cores):
        m = dict(shared)
        # pack the shard so block b is [128, DC*M_BLK] with 32 KiB of
        # contiguous bytes per partition: pk[p, ((b*DC+c)*M_BLK+m)] =
        # x[b*M_BLK+m, c*128+p]
        xs = x2[c * Mc:(c + 1) * Mc, :]
        pk = xs.reshape(NB, M_BLK, DC, 128).transpose(3, 0, 2, 1)
        m["xt"] = np.ascontiguousarray(
            pk.reshape(128, NB * DC * M_BLK).astype(f16))
        in_maps.append(m)
    return in_maps


_NC_CACHE = {}


def kernel(x, lora_A, lora_B, hidden_B, W_gate, b_gate, W_tau, b_tau):
    from concourse.bass_utils import run_bass_kernel_spmd

    global LAST_RESULTS
    key = "main"
    if key not in _NC_CACHE:
        _NC_CACHE[key] = build_nc(D_, O_, M_CORE, R_)
    nc = _NC_CACHE[key]

    in_maps = make_host_inputs(x, lora_A, lora_B, hidden_B,
                               W_gate, b_gate, W_tau, b_tau)
    res = run_bass_kernel_spmd(nc, in_maps, core_ids=list(range(N_CORES)))
    LAST_RESULTS = res
    outs = [np.asarray(res.results[c]["out"]) for c in range(N_CORES)]
    full = np.concatenate(outs, axis=0).astype(np.float32).reshape(B_, S_, O_)
    return np.ascontiguousarray(full)
